# revision 23
# baseline (speedup 1.0000x reference)
"""Trainium2 Bass kernel v3 for HME-VideoQA multi-modal attention GRU.

Changes vs v2 baseline (418us):
- Critical-path collective is an AllGather (floor ~4.6us vs AllReduce ~9.7us)
  of [cv|ct|Z] partials; the cross-rank sum happens locally via ONE selector
  matmul that also lands cv/ct directly in column-block layout (replaces the
  strided-unstage + PE transpose).
- GRU h-partials (gh|hWhh|hWb) move to a separate AllReduce issued right
  after h is computed; it completes under the tanh window, off the critical
  path, and its unstage/transposes also hide there.
- ACT queue carries activations only; all per-iteration DMA triggers moved
  to vector/gpsimd queues (they were serializing with tanh on the scalar
  queue).
- ub/gi row->column conversion via fold-DMA + one PE transpose instead of
  4-8 full 128x128 PE transposes + strided DVE copies.
- gi GEMV rebalanced to 8 chains of N=384 (2 per PE column group).
- PE kept warm through tanh/AllGather windows with dummy matmuls (cold
  matmuls measured 2x slower; HAM re-throttles after ~3.4us idle).
- Startup: all weight loads on one sync ring in need order; small constants
  packed into two tiles; warmup collectives absorb first-call premium and
  cross-core skew under the setup GEMM; bav/bat folded into mvw/mtw.
"""

import numpy as np
import ml_dtypes
from contextlib import ExitStack

H = 1024
P = 128
NCORES = 8
KB = H // P             # 8 H-blocks
TVC = 8192 // NCORES    # 1024 video slots/core
TTC = 2048 // NCORES    # 256 text slots/core
SVB = TVC // P          # 8 video slot blocks
STB = TTC // P          # 2 text slot blocks

# constsF (f32) column offsets
CF_BAV, CF_BAT, CF_BHH, CF_GB = 0, 8, 16, 24
CF_SEL, CF_EYE, CF_ONER, CF_ONE8 = 56, 72, 104, 232
CF_BB, CF_BETA0 = 233, 235
NF = 240
# constsB (bf16) column offsets
CB_VAV, CB_VAT, CB_EYE, CB_MASK = 0, 8, 16, 48
NB = 56

# AR_a payload (f32): [gh 3H | hWhh H | hWb 2 | pad]
A_GH, A_HW, A_WB, A_LEN = 0, 3072, 4096, 4104
# AG_b payload (f32) per rank: [cv|ct 2048 | Zva Zvb Zt | pad]
B_Z, B_LEN = 2048, 2056

DUMN = 20               # dummy matmuls bridging the AllGather window
GI_N = 384              # gi GEMV chain width (8 chains, 2 per col group)

_cache = {}


def _build(loop_n):
    import concourse.bacc as bacc
    import concourse.mybir as mybir
    import concourse.tile as tile
    import concourse.bass as bass  # noqa: F401

    nc = bacc.Bacc("TRN2", target_bir_lowering=False, debug=False,
                   num_devices=NCORES)
    f32 = mybir.dt.float32
    bf16 = mybir.dt.bfloat16
    fp8 = mybir.dt.float8e4
    AF = mybir.ActivationFunctionType
    ALU = mybir.AluOpType
    RG = [list(range(NCORES))]

    def din(name, shape, dty):
        return nc.dram_tensor(name, list(shape), dty,
                              kind="ExternalInput").ap()

    memTv_in = din("memTv", [P, KB * TVC], fp8)
    memTt_in = din("memTt", [P, KB * TTC], fp8)
    memRv_in = din("memRv", [P, SVB * H], bf16)
    memRt_in = din("memRt", [P, STB * H], bf16)
    wavT_in = din("wavT", [P, KB * H], fp8)
    watT_in = din("watT", [P, KB * H], fp8)
    uavR_in = din("uavR", [P, KB * H], fp8)
    uatR_in = din("uatR", [P, KB * H], fp8)
    wvhR_in = din("wvhR", [P, KB * H], bf16)
    wthR_in = din("wthR", [P, KB * H], bf16)
    wihTR_in = din("wihTR", [P, KB * 3 * H], bf16)
    whhTs_in = din("whhTs", [P, 4 * H + 2], bf16)
    constsF_in = din("constsF", [P, NF], f32)
    constsB_in = din("constsB", [P, NB], bf16)
    h_out = nc.dram_tensor("h_out", [P, KB], f32, kind="ExternalOutput").ap()

    with tile.TileContext(nc) as tc, ExitStack() as ctx:
        cst = ctx.enter_context(tc.tile_pool(name="cst", bufs=1))
        wgt = ctx.enter_context(tc.tile_pool(name="wgt", bufs=1))
        res = ctx.enter_context(tc.tile_pool(name="res", bufs=1))
        dram = ctx.enter_context(tc.tile_pool(name="dram", bufs=2,
                                              space="DRAM"))
        pbig = ctx.enter_context(tc.tile_pool(name="pbig", bufs=4,
                                              space="PSUM"))
        psm = ctx.enter_context(tc.tile_pool(name="psm", bufs=2,
                                             space="PSUM"))
        pjk = ctx.enter_context(tc.tile_pool(name="pjk", bufs=1,
                                             space="PSUM"))
        thp = ctx.enter_context(tc.tile_pool(name="thp", bufs=2))
        wk = ctx.enter_context(tc.tile_pool(name="wk", bufs=1))
        hhp = ctx.enter_context(tc.tile_pool(name="hh", bufs=2))
        stg = ctx.enter_context(tc.tile_pool(name="stg", bufs=3))

        # ---- startup loads: ONE sync ring, strict need order ----
        constsB = cst.tile([P, NB], bf16, tag="cB", name="constsB")
        nc.sync.dma_start(constsB[:], constsB_in)
        constsF = cst.tile([P, NF], f32, tag="cF", name="constsF")
        nc.sync.dma_start(constsF[:], constsF_in)
        memT = wgt.tile([P, KB, TVC], fp8, tag="memT", name="memT")
        nc.sync.dma_start(memT[:], memTv_in)

        # handy const APs
        bavB = constsF[:, CF_BAV:CF_BAV + 8]
        batB = constsF[:, CF_BAT:CF_BAT + 8]
        bhhB = constsF[:, CF_BHH:CF_BHH + 8]
        gb_rz = constsF[:, CF_GB:CF_GB + 16]
        gb_in = constsF[:, CF_GB + 16:CF_GB + 24]
        gb_hn = constsF[:, CF_GB + 24:CF_GB + 32]
        selF = constsF[:, CF_SEL:CF_SEL + 16]
        ones1p = constsF[0:1, CF_ONER:CF_ONER + P]
        ones8c = constsF[0:8, CF_ONE8:CF_ONE8 + 1]
        bbS = constsF[0:1, CF_BB:CF_BB + 2]
        beta0 = constsF[0:1, CF_BETA0:CF_BETA0 + 2]
        vavB = constsB[:, CB_VAV:CB_VAV + 8]
        vatB = constsB[:, CB_VAT:CB_VAT + 8]
        maskB = constsB[:, CB_MASK:CB_MASK + 8]

        def eyeF(n):
            return constsF[0:n, CF_EYE:CF_EYE + n]

        def eyeB(n):
            return constsB[0:n, CB_EYE:CB_EYE + n]

        def mm(out, lhsT, rhs, tp, start, stop):
            nc.tensor.matmul(out, lhsT, rhs, start=start, stop=stop,
                             tile_position=tp, skip_group_check=True)

        mvw = res.tile([P, KB * TVC], bf16, tag="mvw", name="mvw")
        mtw = res.tile([P, KB * TTC], bf16, tag="mtw", name="mtw")

        # ---- setup GEMM (video, fp8 DoubleRow: 2 k-blocks per MM) ----
        DR = mybir.MatmulPerfMode.DoubleRow
        for jh in range(2):
            wv = wgt.tile([P, KB, 512], fp8, tag="wtag", name="wv",
                          bufs=2)
            nc.sync.dma_start(wv[:], wavT_in[:, jh * 4096:(jh + 1) * 4096])
            for jj in range(4):
                jb = jh * 4 + jj
                for pc in range(2):
                    ps = pbig.tile([P, 512], f32, tag="big", name="ps")
                    for kp in range(KB // 2):
                        nc.tensor.matmul(
                            ps[:],
                            wv[:, 2 * kp:2 * kp + 2, jj * P:(jj + 1) * P],
                            memT[:, 2 * kp:2 * kp + 2,
                                 pc * 512:(pc + 1) * 512],
                            start=(kp == 0), stop=(kp == KB // 2 - 1),
                            perf_mode=DR)
                    nc.vector.tensor_scalar(
                        mvw[:, jb * TVC + pc * 512: jb * TVC + (pc + 1) * 512],
                        ps[:], 1.0 / 64.0, bavB[:, jb:jb + 1],
                        op0=ALU.mult, op1=ALU.add)

        # text GEMM (fp8 DoubleRow)
        memTtS = wgt.tile([P, KB, TTC], fp8, tag="memTt2", name="memTtS")
        nc.sync.dma_start(memTtS[:], memTt_in)
        for jh in range(2):
            wt = wgt.tile([P, KB, 512], fp8, tag="wtag", name="wt",
                          bufs=2)
            nc.sync.dma_start(wt[:], watT_in[:, jh * 4096:(jh + 1) * 4096])
            for jj in range(4):
                jb = jh * 4 + jj
                ps = pbig.tile([P, TTC], f32, tag="big", name="ps")
                for kp in range(KB // 2):
                    nc.tensor.matmul(
                        ps[:],
                        wt[:, 2 * kp:2 * kp + 2, jj * P:(jj + 1) * P],
                        memTtS[:, 2 * kp:2 * kp + 2, 0:TTC],
                        start=(kp == 0), stop=(kp == KB // 2 - 1),
                        perf_mode=DR)
                nc.vector.tensor_scalar(mtw[:, jb * TTC:(jb + 1) * TTC],
                                        ps[:], 1.0 / 64.0,
                                        batB[:, jb:jb + 1],
                                        op0=ALU.mult, op1=ALU.add)

        # remaining weights, in first-need order (sync ring)
        memRv = wgt.tile([P, SVB * H], bf16, tag="memRv", name="memRv")
        nc.sync.dma_start(memRv[:], memRv_in)
        memRt = wgt.tile([P, STB * H], bf16, tag="memRt", name="memRt")
        nc.sync.dma_start(memRt[:], memRt_in)
        wvhR = wgt.tile([P, KB * H], bf16, tag="wvhR", name="wvhR")
        nc.sync.dma_start(wvhR[:], wvhR_in)
        wthR = wgt.tile([P, KB * H], bf16, tag="wthR", name="wthR")
        nc.sync.dma_start(wthR[:], wthR_in)
        wihTR = wgt.tile([P, KB * 3 * H], bf16, tag="wihTR", name="wihTR")
        nc.sync.dma_start(wihTR[:], wihTR_in)
        uavR = wgt.tile([P, KB * H], fp8, tag="uavR", name="uavR")
        nc.sync.dma_start(uavR[:], uavR_in)
        uatR = wgt.tile([P, KB * H], fp8, tag="uatR", name="uatR")
        nc.sync.dma_start(uatR[:], uatR_in)
        whhTs = wgt.tile([P, 4 * H + 2], bf16, tag="whhTs", name="whhTs")
        nc.sync.dma_start(whhTs[:], whhTs_in)

        jnk = pjk.tile([P, 512], f32, tag="jnk", name="jnk")

        def dummy():
            nc.tensor.matmul(jnk[0:1, :], vavB[:, 0:1], mvw[:, 0:512],
                             start=True, stop=True, tile_position=(0, 0),
                             skip_group_check=True)

        # ---- recurrence ----
        hC = None     # [P, KB] f32, full h, col-block layout
        hB = None     # bf16 copy
        hB8 = None    # fp8 copy

        for it in range(loop_n):
            first = (it == 0)
            last = (it == loop_n - 1)

            if not first:
                # -- h-select for the sharded Whh GEMVs (vector) --
                msk = wk.tile([P, KB], f32, tag="msk", name="msk")
                nc.vector.tensor_tensor(msk[:], hB[:], maskB, op=ALU.mult)
                hsel = wk.tile([P, 1], f32, tag="hsel", name="hsel")
                nc.vector.tensor_reduce(hsel[:], msk[:],
                                        axis=mybir.AxisListType.XYZW,
                                        op=ALU.add)
                hselB = wk.tile([P, 1], bf16, tag="hselB", name="hselB")
                nc.vector.tensor_copy(hselB[:], hsel[:])

                # -- hu GEMV (fp8, replicated): 4 chains x 8 rounds --
                g1 = pbig.tile([P, 512], f32, tag="big", name="g1")
                rhs_map = [(uavR, 0), (uavR, 512), (uatR, 0), (uatR, 512)]
                for kb in range(KB):
                    hcol = hB8[:, kb:kb + 1]
                    for j, (w, off) in enumerate(rhs_map):
                        mm(g1[32 * j:32 * j + 1, :], hcol,
                           w[:, kb * H + off: kb * H + off + 512],
                           (0, 32 * j), kb == 0, kb == KB - 1)

                # -- sharded partials (PE, fills the hu-fold latency) --
                g3 = pbig.tile([P, 512], f32, tag="big", name="g3")
                g4 = pbig.tile([P, 512], f32, tag="big", name="g4")
                for j in range(4):
                    mm(g3[32 * j:32 * j + 1, :], hselB[:],
                       whhTs[:, j * 512:(j + 1) * 512],
                       (0, 32 * j), True, True)
                for j in range(4):
                    mm(g4[32 * j:32 * j + 1, :], hselB[:],
                       whhTs[:, (4 + j) * 512:(5 + j) * 512],
                       (0, 32 * j), True, True)
                pwb = psm.tile([P, 32], f32, tag="smF", name="pwb")
                mm(pwb[0:1, 0:2], hselB[:], whhTs[:, 4 * H:4 * H + 2],
                   (0, 0), True, True)

                # -- hu fold -> column bias (vector-queue DMA + PE tr) --
                sg1 = stg.tile([P, 512], f32, tag="stg", name="sg1")
                nc.vector.tensor_copy(sg1[:], g1[:])
                huF = wk.tile([2 * KB, P], f32, tag="huF", name="huF")
                nc.sync.dma_start(huF[:], sg1[0:128:32, :])
                dummy()
                dummy()
                pt = psm.tile([P, 32], f32, tag="smF", name="pt")
                nc.tensor.transpose(pt[:, 0:2 * KB], huF[:], eyeF(2 * KB))
                huC = wk.tile([P, 2 * KB], f32, tag="huC", name="huC")
                nc.vector.tensor_scalar_mul(huC[:], pt[:, 0:2 * KB],
                                            1.0 / 64.0)

                # -- stage + AllReduce the h-partials (gpsimd; hidden) --
                sg3 = stg.tile([P, 512], f32, tag="stg", name="sg3")
                nc.vector.tensor_copy(sg3[:], g3[:])
                sg4 = stg.tile([P, 512], f32, tag="stg", name="sg4")
                nc.vector.tensor_copy(sg4[:], g4[:])
                spwb = wk.tile([1, 2], f32, tag="spwb", name="spwb")
                nc.vector.tensor_copy(spwb[:], pwb[0:1, 0:2])
                arina = dram.tile([1, A_LEN], f32, tag="arina", name="arina")
                nc.gpsimd.dma_start(arina[0, A_GH:A_GH + 2048],
                                    sg3[0:128:32, :])
                nc.gpsimd.dma_start(arina[0, A_GH + 2048:A_GH + 4096],
                                    sg4[0:128:32, :])
                nc.gpsimd.dma_start(arina[0, A_WB:A_WB + 2], spwb[:])
                arouta = dram.tile([1, A_LEN], f32, tag="arouta",
                                   name="arouta", addr_space="Shared")
                nc.gpsimd.collective_compute(
                    "AllReduce", ALU.add, replica_groups=RG,
                    ins=[arina.opt()], outs=[arouta.opt()])
                ghF = wk.tile([3 * KB, P], f32, tag="ghF", name="ghF")
                nc.gpsimd.dma_start(ghF[:], arouta[0, A_GH:A_GH + 3 * H])
                hWhhF = wk.tile([KB, P], f32, tag="hWhhF", name="hWhhF")
                nc.gpsimd.dma_start(hWhhF[:], arouta[0, A_HW:A_HW + H])
                hwbS = wk.tile([1, 2], f32, tag="hwbS", name="hwbS")
                nc.gpsimd.dma_start(hwbS[:], arouta[0, A_WB:A_WB + 2])

            # --- video tanh + scores (PE kept busy with dummies) ---
            sc = pbig.tile([P, 512], f32, tag="big", name="sc")
            for kb in range(KB):
                th = thp.tile([P, TVC], bf16, tag="thv", name="th")
                bias = 0.0 if first else huC[:, kb:kb + 1]
                nc.scalar.activation(th[:], mvw[:, kb * TVC:(kb + 1) * TVC],
                                     AF.Tanh, bias=bias)
                mm(sc[0:1, :], vavB[:, kb:kb + 1], th[:, 0:512],
                   (0, 0), kb == 0, kb == KB - 1)
                mm(sc[32:33, :], vavB[:, kb:kb + 1], th[:, 512:1024],
                   (0, 32), kb == 0, kb == KB - 1)
                if kb < KB - 1:
                    dummy()

            # --- video exp (+accum; accZ rows 0,32 video / 64 text) ---
            evS = wk.tile([33, 512], bf16, tag="evS", name="evS")
            accZ = wk.tile([65, 1], f32, tag="accZ", name="accZ")
            nc.scalar.activation(evS[:], sc[0:33, :], AF.Exp,
                                 accum_out=accZ[0:33, 0:1])
            evF = wk.tile([SVB, P], bf16, tag="evF", name="evF")
            nc.sync.dma_start(evF[:], evS[0:33:32, :])

            # --- text tanh (ACT continues back-to-back) ---
            thts = []
            for kb in range(KB):
                tht = thp.tile([P, TTC], bf16, tag="tht", name="tht",
                               bufs=KB)
                bias = 0.0 if first else huC[:, KB + kb:KB + kb + 1]
                nc.scalar.activation(tht[:], mtw[:, kb * TTC:(kb + 1) * TTC],
                                     AF.Tanh, bias=bias)
                thts.append(tht)

            # --- video context (PE queue: before text scores) ---
            ptev = psm.tile([P, 32], bf16, tag="smF", name="ptev")
            nc.tensor.transpose(ptev[:, 0:SVB], evF[:], eyeB(SVB))
            evT = wk.tile([P, SVB], bf16, tag="evT", name="evT")
            nc.vector.tensor_copy(evT[:], ptev[:, 0:SVB])
            cx = pbig.tile([P, 512], f32, tag="big", name="cx")
            for sb in range(SVB):
                mm(cx[0:1, :], evT[:, sb:sb + 1],
                   memRv[:, sb * H: sb * H + 512],
                   (0, 0), sb == 0, sb == SVB - 1)
                mm(cx[32:33, :], evT[:, sb:sb + 1],
                   memRv[:, sb * H + 512: (sb + 1) * H],
                   (0, 32), sb == 0, sb == SVB - 1)

            # stage the video half early (overlaps the text phase)
            arinb = dram.tile([1, B_LEN], f32, tag="arinb", name="arinb")
            scxv = stg.tile([33, 512], f32, tag="scxv", name="scxv", bufs=1)
            nc.vector.tensor_copy(scxv[:], cx[0:33, :])
            nc.gpsimd.dma_start(arinb[0, 0:1024], scxv[0:33:32, :])

            # --- text scores + exp + context ---
            for kb in range(KB):
                mm(sc[64:65, 0:TTC], vatB[:, kb:kb + 1], thts[kb][:],
                   (0, 64), kb == 0, kb == KB - 1)
            etS = wk.tile([1, TTC], bf16, tag="etS", name="etS")
            nc.scalar.activation(etS[:], sc[64:65, 0:TTC], AF.Exp,
                                 accum_out=accZ[64:65, 0:1])
            etF = wk.tile([STB, P], bf16, tag="etF", name="etF")
            nc.sync.dma_start(etF[:], etS[:])

            ptet = psm.tile([P, 32], bf16, tag="smF", name="ptet")
            nc.tensor.transpose(ptet[:, 0:STB], etF[:], eyeB(STB))
            etT = wk.tile([P, STB], bf16, tag="etT", name="etT")
            nc.vector.tensor_copy(etT[:], ptet[:, 0:STB])
            for sb in range(STB):
                mm(cx[64:65, :], etT[:, sb:sb + 1],
                   memRt[:, sb * H: sb * H + 512],
                   (0, 64), sb == 0, sb == STB - 1)
                mm(cx[96:97, :], etT[:, sb:sb + 1],
                   memRt[:, sb * H + 512: (sb + 1) * H],
                   (0, 96), sb == 0, sb == STB - 1)

            # --- stage text half + Z (gpsimd DMAs) ---
            scxt = stg.tile([97, 512], f32, tag="scxt", name="scxt", bufs=1)
            nc.vector.tensor_copy(scxt[64:97, :], cx[64:97, :])
            nc.gpsimd.dma_start(arinb[0, 1024:2048], scxt[64:97:32, :])
            nc.gpsimd.dma_start(arinb[0, B_Z:B_Z + 3], accZ[0:65:32, 0:1])

            # --- gh unstage transposes + beta chain (hidden window) ---
            if not first:
                ptgh = psm.tile([P, 32], f32, tag="smF", name="ptgh")
                nc.tensor.transpose(ptgh[:, 0:3 * KB], ghF[:], eyeF(3 * KB))
                ptW = psm.tile([P, 32], f32, tag="smF", name="ptW")
                nc.tensor.transpose(ptW[:, 0:KB], hWhhF[:], eyeF(KB))
                ghCrz = wk.tile([P, 2 * KB], f32, tag="ghCrz", name="ghCrz")
                nc.vector.tensor_tensor(ghCrz[:], ptgh[:, 0:2 * KB], gb_rz,
                                        op=ALU.add)
                hnB = wk.tile([P, KB], f32, tag="hnB", name="hnB")
                nc.vector.tensor_tensor(hnB[:], ptgh[:, 2 * KB:3 * KB],
                                        gb_hn, op=ALU.add)
                hwbC = wk.tile([P, KB], f32, tag="hwbC", name="hwbC")
                nc.vector.tensor_tensor(hwbC[:], ptW[:, 0:KB], bhhB,
                                        op=ALU.add)
                bsum = wk.tile([1, 2], f32, tag="bsum", name="bsum")
                nc.vector.tensor_tensor(bsum[:], hwbS[:], bbS, op=ALU.add)
                eb = wk.tile([1, 2], f32, tag="eb", name="eb")
                ebs = wk.tile([1, 1], f32, tag="ebs", name="ebs")
                nc.scalar.activation(eb[:], bsum[:], AF.Exp, accum_out=ebs[:])
                erec = wk.tile([1, 1], f32, tag="erec", name="erec")
                nc.vector.reciprocal(erec[:], ebs[:])
                beta = wk.tile([1, 2], f32, tag="beta", name="beta")
                nc.vector.tensor_scalar_mul(beta[:], eb[:], erec[:])
                beta_ap = beta[:]
                ghCrz_ap, hnB_ap, hwbC_ap = ghCrz[:], hnB[:], hwbC[:]
            else:
                beta_ap = beta0
                ghCrz_ap, hnB_ap, hwbC_ap = gb_rz, gb_hn, bhhB

            # --- AllGather [cv|ct|Z] ---
            aroutg = dram.tile([NCORES, B_LEN], f32, tag="aroutg",
                               name="aroutg", addr_space="Shared")
            nc.gpsimd.collective_compute(
                "AllGather", ALU.bypass, replica_groups=RG,
                ins=[arinb.opt()], outs=[aroutg.opt()])

            # Unstage triggers ride the idle scalar queue: it reaches them
            # while the collective is still in flight, so they fire the
            # moment it completes (no post-AG trigger serialization).
            cvfold = wk.tile([P, P], f32, tag="cvfold", name="cvfold")
            nc.scalar.dma_start(cvfold[:], aroutg[0:NCORES, 0:2048])
            zfold = wk.tile([NCORES, 3], f32, tag="zfold", name="zfold")
            nc.scalar.dma_start(zfold[:], aroutg[0:NCORES, B_Z:B_Z + 3])

            # PE: bridge the collective
            for _ in range(DUMN):
                dummy()

            # --- local reduce via selector matmul ---
            zps = psm.tile([P, 32], f32, tag="smF", name="zps")
            nc.tensor.matmul(zps[0:1, 0:3], ones8c, zfold[:],
                             start=True, stop=True, skip_group_check=True)
            cvct = psm.tile([P, 32], f32, tag="smF", name="cvct")
            nc.tensor.matmul(cvct[:, 0:2 * KB], cvfold[:], selF,
                             start=True, stop=True, skip_group_check=True)
            cvctB = wk.tile([P, 2 * KB], bf16, tag="cvctB", name="cvctB")
            nc.vector.tensor_copy(cvctB[:], cvct[:, 0:2 * KB])

            # --- Z / rr scalars (vector) ---
            zS = wk.tile([1, 3], f32, tag="zS", name="zS")
            nc.vector.tensor_copy(zS[:], zps[0:1, 0:3])
            zz = wk.tile([1, 2], f32, tag="zz", name="zz")
            nc.vector.tensor_tensor(zz[:, 0:1], zS[:, 0:1],
                                    zS[:, 1:2], op=ALU.add)
            nc.vector.tensor_copy(zz[:, 1:2], zS[:, 2:3])
            zrec = wk.tile([1, 2], f32, tag="zrec", name="zrec")
            nc.vector.reciprocal(zrec[:], zz[:])
            rr = wk.tile([1, 2], f32, tag="rr", name="rr")
            nc.vector.tensor_tensor(rr[:], beta_ap, zrec[:], op=ALU.mult)

            # --- u GEMV: u = cv @ Wvh, ut = ct @ Wth ---
            ub = pbig.tile([P, 512], f32, tag="big", name="ub")
            for kb in range(KB):
                mm(ub[0:1, :], cvctB[:, kb:kb + 1],
                   wvhR[:, kb * H: kb * H + 512],
                   (0, 0), kb == 0, kb == KB - 1)
                mm(ub[32:33, :], cvctB[:, kb:kb + 1],
                   wvhR[:, kb * H + 512: (kb + 1) * H],
                   (0, 32), kb == 0, kb == KB - 1)
                mm(ub[64:65, :], cvctB[:, KB + kb:KB + kb + 1],
                   wthR[:, kb * H: kb * H + 512],
                   (0, 64), kb == 0, kb == KB - 1)
                mm(ub[96:97, :], cvctB[:, KB + kb:KB + kb + 1],
                   wthR[:, kb * H + 512: (kb + 1) * H],
                   (0, 96), kb == 0, kb == KB - 1)

            # rr broadcast over partitions (PE; fills the ub-flush gap)
            prr = psm.tile([P, 32], f32, tag="smF", name="prr")
            nc.tensor.matmul(prr[:, 0:2], ones1p, rr[:],
                             start=True, stop=True, skip_group_check=True)
            rrB = wk.tile([P, 2], f32, tag="rrB", name="rrB")
            nc.vector.tensor_copy(rrB[:], prr[:, 0:2])

            # --- ub fold -> columns; mm_o ---
            sub = stg.tile([P, 512], f32, tag="stg", name="sub")
            nc.vector.tensor_copy(sub[:], ub[:])
            ubF = wk.tile([2 * KB, P], f32, tag="ubF", name="ubF")
            nc.sync.dma_start(ubF[:], sub[0:128:32, :])
            dummy()
            dummy()
            ptU = psm.tile([P, 32], f32, tag="smF", name="ptU")
            nc.tensor.transpose(ptU[:, 0:2 * KB], ubF[:], eyeF(2 * KB))
            t1 = wk.tile([P, KB], f32, tag="t1", name="t1")
            nc.vector.scalar_tensor_tensor(t1[:], ptU[:, 0:KB], rrB[:, 0:1],
                                           hwbC_ap, op0=ALU.mult,
                                           op1=ALU.add)
            t2 = wk.tile([P, KB], f32, tag="t2", name="t2")
            nc.vector.scalar_tensor_tensor(t2[:], ptU[:, KB:2 * KB],
                                           rrB[:, 1:2], t1[:],
                                           op0=ALU.mult, op1=ALU.add)
            moB = wk.tile([P, KB], bf16, tag="moB", name="moB")
            nc.scalar.activation(moB[:], t2[:], AF.Tanh)

            # --- gi GEMV: gi = mo @ W_ih.T (8 chains of N=384) ---
            giE = pbig.tile([P, 512], f32, tag="big", name="giE")
            giF_ = pbig.tile([P, 512], f32, tag="big", name="giF_")
            for kb in range(KB):
                mo_col = moB[:, kb:kb + 1]
                base = kb * 3 * H
                for c in range(4):
                    mm(giE[32 * c:32 * c + 1, 0:GI_N], mo_col,
                       wihTR[:, base + c * GI_N: base + (c + 1) * GI_N],
                       (0, 32 * c), kb == 0, kb == KB - 1)
                for c in range(4):
                    mm(giF_[32 * c:32 * c + 1, 0:GI_N], mo_col,
                       wihTR[:, base + (4 + c) * GI_N:
                             base + (5 + c) * GI_N],
                       (0, 32 * c), kb == 0, kb == KB - 1)

            # gi fold -> columns [128, 24]
            sgiE = stg.tile([P, 512], f32, tag="stg", name="sgiE")
            nc.vector.tensor_copy(sgiE[:], giE[:])
            sgiF = stg.tile([P, 512], f32, tag="stg", name="sgiF")
            nc.vector.tensor_copy(sgiF[:], giF_[:])
            giFold = wk.tile([3 * KB, P], f32, tag="giFold", name="giFold")
            nc.sync.dma_start(giFold[0:12, :], sgiE[0:128:32, 0:GI_N])
            nc.sync.dma_start(giFold[12:24, :], sgiF[0:128:32, 0:GI_N])
            for _ in range(5):
                dummy()
            ptgi = psm.tile([P, 32], f32, tag="smF", name="ptgi")
            nc.tensor.transpose(ptgi[:, 0:3 * KB], giFold[:], eyeF(3 * KB))

            # --- gates (columns; r 0-7, z 8-15, n 16-23) ---
            pre = wk.tile([P, 2 * KB], f32, tag="pre", name="pre")
            nc.vector.tensor_tensor(pre[:], ptgi[:, 0:2 * KB], ghCrz_ap,
                                    op=ALU.add)
            tnB = wk.tile([P, KB], f32, tag="tnB", name="tnB")
            nc.vector.tensor_tensor(tnB[:], ptgi[:, 2 * KB:3 * KB], gb_in,
                                    op=ALU.add)
            # sigmoid(x) = 0.5*tanh(0.5x) + 0.5 (tanh is in the exp table set)
            trz = wk.tile([P, 2 * KB], f32, tag="trz", name="trz")
            nc.scalar.activation(trz[:], pre[:], AF.Tanh, scale=0.5)
            rz = wk.tile([P, 2 * KB], f32, tag="rz", name="rz")
            nc.vector.tensor_scalar(rz[:], trz[:], 0.5, 0.5,
                                    op0=ALU.mult, op1=ALU.add)
            m1 = wk.tile([P, KB], f32, tag="m1", name="m1")
            nc.vector.tensor_tensor(m1[:], rz[:, 0:KB], hnB_ap, op=ALU.mult)
            tn = wk.tile([P, KB], f32, tag="tn", name="tn")
            nc.vector.tensor_tensor(tn[:], tnB[:], m1[:], op=ALU.add)
            ng = wk.tile([P, KB], f32, tag="ng", name="ng")
            nc.scalar.activation(ng[:], tn[:], AF.Tanh)
            hC_new = hhp.tile([P, KB], f32, tag="hC", name="hC")
            d = wk.tile([P, KB], f32, tag="d", name="d")
            if first:
                nc.vector.tensor_tensor(d[:], rz[:, KB:2 * KB], ng[:],
                                        op=ALU.mult)
                nc.vector.tensor_tensor(hC_new[:], ng[:], d[:],
                                        op=ALU.subtract)
            else:
                nc.vector.tensor_tensor(d[:], hC[:], ng[:], op=ALU.subtract)
                zd = wk.tile([P, KB], f32, tag="zd", name="zd")
                nc.vector.tensor_tensor(zd[:], rz[:, KB:2 * KB], d[:],
                                        op=ALU.mult)
                nc.vector.tensor_tensor(hC_new[:], ng[:], zd[:], op=ALU.add)
            hC = hC_new
            if not last:
                hB_new = hhp.tile([P, KB], bf16, tag="hB", name="hB")
                nc.vector.tensor_copy(hB_new[:], hC[:])
                hB = hB_new
                hB8_new = hhp.tile([P, KB], fp8, tag="hB8", name="hB8")
                nc.vector.tensor_copy(hB8_new[:], hC[:])
                hB8 = hB8_new

        nc.sync.dma_start(h_out, hC[:])

    nc.compile()
    return nc


def _bf(x):
    return np.ascontiguousarray(np.asarray(x, dtype=ml_dtypes.bfloat16))


def _f8(x):
    return np.ascontiguousarray(np.asarray(x, dtype=ml_dtypes.float8_e4m3))


def _f32(x):
    return np.ascontiguousarray(np.asarray(x, dtype=np.float32))


def _kblocks(W):
    """[H, N] -> [128, KB*N]: block kb = W[kb*128:(kb+1)*128, :]."""
    N = W.shape[1]
    return np.ascontiguousarray(
        W.reshape(KB, P, N).transpose(1, 0, 2).reshape(P, KB * N))


def _halfpack(W):
    """[H, H] -> [128, 2*KB*512]: half jh, block kb = W[kb-rows, jh-cols]."""
    X = W.reshape(KB, P, 2, 512)           # [kb, p, jh, 512]
    return np.ascontiguousarray(
        X.transpose(1, 2, 0, 3).reshape(P, 2 * KB * 512))


def _memT_blk(M):
    """[T, H] -> [128, KB*T]: block kb holds M.T[kb*128:(kb+1)*128, :]."""
    T = M.shape[0]
    X = np.ascontiguousarray(M.T)
    return np.ascontiguousarray(
        X.reshape(KB, P, T).transpose(1, 0, 2).reshape(P, KB * T))


def _colblk(v):
    return np.ascontiguousarray(v.reshape(KB, P).T)


def _prep_inputs(inputs):
    mem_v = _f32(inputs["memory_vid"])
    mem_t = _f32(inputs["memory_text"])
    Wav, Uav, bav, Vav = (_f32(inputs[k]) for k in ("Wav", "Uav", "bav", "Vav"))
    Wat, Uat, bat, Vat = (_f32(inputs[k]) for k in ("Wat", "Uat", "bat", "Vat"))
    Wb, bb = _f32(inputs["Wb"]), _f32(inputs["bb"])
    Whh, Wvh, Wth, bhh = (_f32(inputs[k]) for k in ("Whh", "Wvh", "Wth", "bhh"))
    W_ih, W_hh = _f32(inputs["W_ih"]), _f32(inputs["W_hh"])
    b_ih, b_hh = _f32(inputs["b_ih"]), _f32(inputs["b_hh"])

    wavT_b = _f8(_halfpack(Wav * 64.0))
    watT_b = _f8(_halfpack(Wat * 64.0))
    uavR_b = _f8(_kblocks(Uav * 64.0))
    uatR_b = _f8(_kblocks(Uat * 64.0))
    wvhR_b = _bf(_kblocks(Wvh))
    wthR_b = _bf(_kblocks(Wth))
    wihTR_b = _bf(_kblocks(np.ascontiguousarray(W_ih.T)))

    # constsF
    constsF = np.zeros((P, NF), np.float32)
    constsF[:, CF_BAV:CF_BAV + 8] = _colblk(bav)
    constsF[:, CF_BAT:CF_BAT + 8] = _colblk(bat)
    constsF[:, CF_BHH:CF_BHH + 8] = _colblk(bhh)
    constsF[:, CF_GB:CF_GB + 32] = np.concatenate([
        _colblk(b_ih[0:H] + b_hh[0:H]),
        _colblk(b_ih[H:2 * H] + b_hh[H:2 * H]),
        _colblk(b_ih[2 * H:3 * H]),
        _colblk(b_hh[2 * H:3 * H]),
    ], axis=1)
    sel = np.zeros((P, 16), np.float32)
    for p in range(P):
        sel[p, p % 16] = 1.0
    constsF[:, CF_SEL:CF_SEL + 16] = sel
    constsF[0:32, CF_EYE:CF_EYE + 32] = np.eye(32, dtype=np.float32)
    constsF[:, CF_ONER:CF_ONER + P] = 1.0
    constsF[:, CF_ONE8:CF_ONE8 + 1] = 1.0
    constsF[0, CF_BB:CF_BB + 2] = bb
    ebb = np.exp(bb - bb.max())
    constsF[0, CF_BETA0:CF_BETA0 + 2] = ebb / ebb.sum()

    # constsB (maskB is per-core, added below)
    constsB = np.zeros((P, NB), np.float32)
    constsB[:, CB_VAV:CB_VAV + 8] = _colblk(Vav)
    constsB[:, CB_VAT:CB_VAT + 8] = _colblk(Vat)
    constsB[0:32, CB_EYE:CB_EYE + 32] = np.eye(32, dtype=np.float32)

    in_maps = []
    for c in range(NCORES):
        svc = slice(c * TVC, (c + 1) * TVC)
        stc = slice(c * TTC, (c + 1) * TTC)
        cslice = slice(c * P, (c + 1) * P)
        mv_c, mt_c = mem_v[svc], mem_t[stc]
        memRv_b = _bf(mv_c.reshape(SVB, P, H).transpose(1, 0, 2)
                      .reshape(P, SVB * H))
        memRt_b = _bf(mt_c.reshape(STB, P, H).transpose(1, 0, 2)
                      .reshape(P, STB * H))
        whhTs = np.concatenate(
            [np.ascontiguousarray(W_hh[:, cslice].T),   # [128, 3H]
             np.ascontiguousarray(Whh[cslice, :]),      # [128, H]
             np.ascontiguousarray(Wb[cslice, :])], axis=1)
        cB = constsB.copy()
        cB[:, CB_MASK + c] = 1.0
        in_maps.append({
            "memTv": _f8(_memT_blk(mv_c)),
            "memTt": _f8(_memT_blk(mt_c)),
            "memRv": memRv_b, "memRt": memRt_b,
            "wavT": wavT_b, "watT": watT_b,
            "uavR": uavR_b, "uatR": uatR_b,
            "wvhR": wvhR_b, "wthR": wthR_b, "wihTR": wihTR_b,
            "whhTs": _bf(whhTs),
            "constsF": constsF, "constsB": _bf(cB),
        })
    return in_maps


TRACE = False
LAST_RESULT = None


def kernel(**inputs):
    global LAST_RESULT
    from concourse import bass_utils
    loop_n = int(np.asarray(inputs["loop"]))
    if loop_n not in _cache:
        _cache[loop_n] = _build(loop_n)
    nc = _cache[loop_n]
    in_maps = _prep_inputs(inputs)
    kw = {}
    if TRACE:
        import tempfile
        kw = dict(trace=True, tmpdir=tempfile.mkdtemp(prefix="bassprof_"))
    res = bass_utils.run_bass_kernel_spmd(nc, in_maps,
                                          core_ids=list(range(NCORES)), **kw)
    LAST_RESULT = res
    hC = res.results[0]["h_out"]  # [128, 8] col-block layout
    h = np.ascontiguousarray(hC.T.reshape(H))
    return h.reshape(1, H).astype(np.float32)


# revision 26
# speedup vs baseline: 1.0024x; 1.0024x over previous
"""Trainium2 Bass kernel v3 for HME-VideoQA multi-modal attention GRU.

Changes vs v2 baseline (418us):
- Critical-path collective is an AllGather (floor ~4.6us vs AllReduce ~9.7us)
  of [cv|ct|Z] partials; the cross-rank sum happens locally via ONE selector
  matmul that also lands cv/ct directly in column-block layout (replaces the
  strided-unstage + PE transpose).
- GRU h-partials (gh|hWhh|hWb) move to a separate AllReduce issued right
  after h is computed; it completes under the tanh window, off the critical
  path, and its unstage/transposes also hide there.
- ACT queue carries activations only; all per-iteration DMA triggers moved
  to vector/gpsimd queues (they were serializing with tanh on the scalar
  queue).
- ub/gi row->column conversion via fold-DMA + one PE transpose instead of
  4-8 full 128x128 PE transposes + strided DVE copies.
- gi GEMV rebalanced to 8 chains of N=384 (2 per PE column group).
- PE kept warm through tanh/AllGather windows with dummy matmuls (cold
  matmuls measured 2x slower; HAM re-throttles after ~3.4us idle).
- Startup: all weight loads on one sync ring in need order; small constants
  packed into two tiles; warmup collectives absorb first-call premium and
  cross-core skew under the setup GEMM; bav/bat folded into mvw/mtw.
"""

import numpy as np
import ml_dtypes
from contextlib import ExitStack

H = 1024
P = 128
NCORES = 8
KB = H // P             # 8 H-blocks
TVC = 8192 // NCORES    # 1024 video slots/core
TTC = 2048 // NCORES    # 256 text slots/core
SVB = TVC // P          # 8 video slot blocks
STB = TTC // P          # 2 text slot blocks

# constsF (f32) column offsets
CF_BAV, CF_BAT, CF_BHH, CF_GB = 0, 8, 16, 24
CF_SEL, CF_EYE, CF_ONER, CF_ONE8 = 56, 72, 104, 232
CF_BB, CF_BETA0 = 233, 235
NF = 240
# constsB (bf16) column offsets
CB_VAV, CB_VAT, CB_EYE, CB_MASK = 0, 8, 16, 48
NB = 56

# AR_a payload (f32): [gh 3H | hWhh H | hWb 2 | pad]
A_GH, A_HW, A_WB, A_LEN = 0, 3072, 4096, 4104
# AG_b payload (f32) per rank: [cv|ct 2048 | Zva Zvb Zt | pad]
B_Z, B_LEN = 2048, 2056

DUMN = 20               # dummy matmuls bridging the AllGather window
GI_N = 384              # gi GEMV chain width (8 chains, 2 per col group)

_cache = {}


def _build(loop_n):
    import concourse.bacc as bacc
    import concourse.mybir as mybir
    import concourse.tile as tile
    import concourse.bass as bass  # noqa: F401

    nc = bacc.Bacc("TRN2", target_bir_lowering=False, debug=False,
                   num_devices=NCORES)
    f32 = mybir.dt.float32
    bf16 = mybir.dt.bfloat16
    fp8 = mybir.dt.float8e4
    AF = mybir.ActivationFunctionType
    ALU = mybir.AluOpType
    RG = [list(range(NCORES))]

    def din(name, shape, dty):
        return nc.dram_tensor(name, list(shape), dty,
                              kind="ExternalInput").ap()

    memTv_in = din("memTv", [P, KB * TVC], bf16)
    memTt_in = din("memTt", [P, KB * TTC], bf16)
    memRv_in = din("memRv", [P, SVB * H], bf16)
    memRt_in = din("memRt", [P, STB * H], bf16)
    wavT_in = din("wavT", [P, KB * H], bf16)
    watT_in = din("watT", [P, KB * H], bf16)
    uavR_in = din("uavR", [P, KB * H], fp8)
    uatR_in = din("uatR", [P, KB * H], fp8)
    wvhR_in = din("wvhR", [P, KB * H], bf16)
    wthR_in = din("wthR", [P, KB * H], bf16)
    wihTR_in = din("wihTR", [P, KB * 3 * H], bf16)
    whhTs_in = din("whhTs", [P, 4 * H + 2], bf16)
    constsF_in = din("constsF", [P, NF], f32)
    constsB_in = din("constsB", [P, NB], bf16)
    h_out = nc.dram_tensor("h_out", [P, KB], f32, kind="ExternalOutput").ap()

    with tile.TileContext(nc) as tc, ExitStack() as ctx:
        cst = ctx.enter_context(tc.tile_pool(name="cst", bufs=1))
        wgt = ctx.enter_context(tc.tile_pool(name="wgt", bufs=1))
        res = ctx.enter_context(tc.tile_pool(name="res", bufs=1))
        dram = ctx.enter_context(tc.tile_pool(name="dram", bufs=2,
                                              space="DRAM"))
        pbig = ctx.enter_context(tc.tile_pool(name="pbig", bufs=4,
                                              space="PSUM"))
        psm = ctx.enter_context(tc.tile_pool(name="psm", bufs=2,
                                             space="PSUM"))
        pjk = ctx.enter_context(tc.tile_pool(name="pjk", bufs=1,
                                             space="PSUM"))
        thp = ctx.enter_context(tc.tile_pool(name="thp", bufs=2))
        wk = ctx.enter_context(tc.tile_pool(name="wk", bufs=1))
        hhp = ctx.enter_context(tc.tile_pool(name="hh", bufs=2))
        stg = ctx.enter_context(tc.tile_pool(name="stg", bufs=3))

        # ---- startup loads: ONE sync ring, strict need order ----
        constsB = cst.tile([P, NB], bf16, tag="cB", name="constsB")
        nc.sync.dma_start(constsB[:], constsB_in)
        constsF = cst.tile([P, NF], f32, tag="cF", name="constsF")
        nc.sync.dma_start(constsF[:], constsF_in)
        memT = wgt.tile([P, KB * TVC], bf16, tag="memT", name="memT")
        nc.sync.dma_start(memT[:], memTv_in)

        # handy const APs
        bavB = constsF[:, CF_BAV:CF_BAV + 8]
        batB = constsF[:, CF_BAT:CF_BAT + 8]
        bhhB = constsF[:, CF_BHH:CF_BHH + 8]
        gb_rz = constsF[:, CF_GB:CF_GB + 16]
        gb_in = constsF[:, CF_GB + 16:CF_GB + 24]
        gb_hn = constsF[:, CF_GB + 24:CF_GB + 32]
        selF = constsF[:, CF_SEL:CF_SEL + 16]
        ones1p = constsF[0:1, CF_ONER:CF_ONER + P]
        ones8c = constsF[0:8, CF_ONE8:CF_ONE8 + 1]
        bbS = constsF[0:1, CF_BB:CF_BB + 2]
        beta0 = constsF[0:1, CF_BETA0:CF_BETA0 + 2]
        vavB = constsB[:, CB_VAV:CB_VAV + 8]
        vatB = constsB[:, CB_VAT:CB_VAT + 8]
        maskB = constsB[:, CB_MASK:CB_MASK + 8]

        def eyeF(n):
            return constsF[0:n, CF_EYE:CF_EYE + n]

        def eyeB(n):
            return constsB[0:n, CB_EYE:CB_EYE + n]

        def mm(out, lhsT, rhs, tp, start, stop):
            nc.tensor.matmul(out, lhsT, rhs, start=start, stop=stop,
                             tile_position=tp, skip_group_check=True)

        # ---- warmup AllGather at the REAL payload size (hidden under the
        # setup GEMM; absorbs the one-time CC bootstrap + cross-core skew
        # so the first real AG runs at steady-state latency) ----
        wsrc = cst.tile([8, B_LEN // 8], f32, tag="wsrc", name="wsrc")
        nc.vector.memset(wsrc[:], 1.0)
        win = dram.tile([1, B_LEN], f32, tag="win", name="win")
        nc.gpsimd.dma_start(win[:], wsrc[:])
        wag = dram.tile([NCORES, B_LEN], f32, tag="wag", name="wag",
                        addr_space="Shared")
        nc.gpsimd.collective_compute(
            "AllGather", ALU.bypass, replica_groups=RG,
            ins=[win.opt()], outs=[wag.opt()])

        mvw = res.tile([P, KB * TVC], bf16, tag="mvw", name="mvw")
        mtw = res.tile([P, KB * TTC], bf16, tag="mtw", name="mtw")

        # ---- setup GEMM (video): mvw[jb] = (mem@Wav).T + bav ----
        for jh in range(2):
            wv = wgt.tile([P, KB * 512], bf16, tag="wtag", name="wv",
                          bufs=2)
            nc.sync.dma_start(wv[:], wavT_in[:, jh * 4096:(jh + 1) * 4096])
            for jj in range(4):
                jb = jh * 4 + jj
                for pc in range(2):
                    ps = pbig.tile([P, 512], f32, tag="big", name="ps")
                    for kb in range(KB):
                        nc.tensor.matmul(
                            ps[:],
                            wv[:, kb * 512 + jj * P: kb * 512 + (jj + 1) * P],
                            memT[:, kb * TVC + pc * 512:
                                 kb * TVC + (pc + 1) * 512],
                            start=(kb == 0), stop=(kb == KB - 1))
                    nc.vector.tensor_scalar_add(
                        mvw[:, jb * TVC + pc * 512: jb * TVC + (pc + 1) * 512],
                        ps[:], bavB[:, jb:jb + 1])

        # text GEMM
        memTtS = wgt.tile([P, KB * TTC], bf16, tag="memTt2", name="memTtS")
        nc.sync.dma_start(memTtS[:], memTt_in)
        for jh in range(2):
            wt = wgt.tile([P, KB * 512], bf16, tag="wtag", name="wt",
                          bufs=2)
            nc.sync.dma_start(wt[:], watT_in[:, jh * 4096:(jh + 1) * 4096])
            for jj in range(4):
                jb = jh * 4 + jj
                ps = pbig.tile([P, TTC], f32, tag="big", name="ps")
                for kb in range(KB):
                    nc.tensor.matmul(
                        ps[:],
                        wt[:, kb * 512 + jj * P: kb * 512 + (jj + 1) * P],
                        memTtS[:, kb * TTC: (kb + 1) * TTC],
                        start=(kb == 0), stop=(kb == KB - 1))
                nc.vector.tensor_scalar_add(mtw[:, jb * TTC:(jb + 1) * TTC],
                                            ps[:], batB[:, jb:jb + 1])

        # remaining weights, in first-need order (sync ring)
        memRv = wgt.tile([P, SVB * H], bf16, tag="memRv", name="memRv")
        nc.sync.dma_start(memRv[:], memRv_in)
        memRt = wgt.tile([P, STB * H], bf16, tag="memRt", name="memRt")
        nc.sync.dma_start(memRt[:], memRt_in)
        wvhR = wgt.tile([P, KB * H], bf16, tag="wvhR", name="wvhR")
        nc.sync.dma_start(wvhR[:], wvhR_in)
        wthR = wgt.tile([P, KB * H], bf16, tag="wthR", name="wthR")
        nc.sync.dma_start(wthR[:], wthR_in)
        wihTR = wgt.tile([P, KB * 3 * H], bf16, tag="wihTR", name="wihTR")
        nc.sync.dma_start(wihTR[:], wihTR_in)
        uavR = wgt.tile([P, KB * H], fp8, tag="uavR", name="uavR")
        nc.sync.dma_start(uavR[:], uavR_in)
        uatR = wgt.tile([P, KB * H], fp8, tag="uatR", name="uatR")
        nc.sync.dma_start(uatR[:], uatR_in)
        whhTs = wgt.tile([P, 4 * H + 2], bf16, tag="whhTs", name="whhTs")
        nc.sync.dma_start(whhTs[:], whhTs_in)

        jnk = pjk.tile([P, 512], f32, tag="jnk", name="jnk")

        def dummy():
            nc.tensor.matmul(jnk[0:1, :], vavB[:, 0:1], mvw[:, 0:512],
                             start=True, stop=True, tile_position=(0, 0),
                             skip_group_check=True)

        # ---- recurrence ----
        hC = None     # [P, KB] f32, full h, col-block layout
        hB = None     # bf16 copy
        hB8 = None    # fp8 copy

        for it in range(loop_n):
            first = (it == 0)
            last = (it == loop_n - 1)

            if not first:
                # -- h-select for the sharded Whh GEMVs (vector) --
                msk = wk.tile([P, KB], f32, tag="msk", name="msk")
                nc.vector.tensor_tensor(msk[:], hB[:], maskB, op=ALU.mult)
                hsel = wk.tile([P, 1], f32, tag="hsel", name="hsel")
                nc.vector.tensor_reduce(hsel[:], msk[:],
                                        axis=mybir.AxisListType.XYZW,
                                        op=ALU.add)
                hselB = wk.tile([P, 1], bf16, tag="hselB", name="hselB")
                nc.vector.tensor_copy(hselB[:], hsel[:])

                # -- hu GEMV (fp8, replicated): 4 chains x 8 rounds --
                g1 = pbig.tile([P, 512], f32, tag="big", name="g1")
                rhs_map = [(uavR, 0), (uavR, 512), (uatR, 0), (uatR, 512)]
                for kb in range(KB):
                    hcol = hB8[:, kb:kb + 1]
                    for j, (w, off) in enumerate(rhs_map):
                        mm(g1[32 * j:32 * j + 1, :], hcol,
                           w[:, kb * H + off: kb * H + off + 512],
                           (0, 32 * j), kb == 0, kb == KB - 1)

                # -- sharded partials (PE, fills the hu-fold latency) --
                g3 = pbig.tile([P, 512], f32, tag="big", name="g3")
                g4 = pbig.tile([P, 512], f32, tag="big", name="g4")
                for j in range(4):
                    mm(g3[32 * j:32 * j + 1, :], hselB[:],
                       whhTs[:, j * 512:(j + 1) * 512],
                       (0, 32 * j), True, True)
                for j in range(4):
                    mm(g4[32 * j:32 * j + 1, :], hselB[:],
                       whhTs[:, (4 + j) * 512:(5 + j) * 512],
                       (0, 32 * j), True, True)
                pwb = psm.tile([P, 32], f32, tag="smF", name="pwb")
                mm(pwb[0:1, 0:2], hselB[:], whhTs[:, 4 * H:4 * H + 2],
                   (0, 0), True, True)

                # -- hu fold -> column bias (vector-queue DMA + PE tr) --
                sg1 = stg.tile([P, 512], f32, tag="stg", name="sg1")
                nc.vector.tensor_copy(sg1[:], g1[:])
                huF = wk.tile([2 * KB, P], f32, tag="huF", name="huF")
                nc.sync.dma_start(huF[:], sg1[0:128:32, :])
                dummy()
                dummy()
                pt = psm.tile([P, 32], f32, tag="smF", name="pt")
                nc.tensor.transpose(pt[:, 0:2 * KB], huF[:], eyeF(2 * KB))
                huC = wk.tile([P, 2 * KB], f32, tag="huC", name="huC")
                nc.vector.tensor_copy(huC[:], pt[:, 0:2 * KB])

                # -- stage + AllReduce the h-partials (gpsimd; hidden) --
                sg3 = stg.tile([P, 512], f32, tag="stg", name="sg3")
                nc.vector.tensor_copy(sg3[:], g3[:])
                sg4 = stg.tile([P, 512], f32, tag="stg", name="sg4")
                nc.vector.tensor_copy(sg4[:], g4[:])
                spwb = wk.tile([1, 2], f32, tag="spwb", name="spwb")
                nc.vector.tensor_copy(spwb[:], pwb[0:1, 0:2])
                arina = dram.tile([1, A_LEN], f32, tag="arina", name="arina")
                nc.gpsimd.dma_start(arina[0, A_GH:A_GH + 2048],
                                    sg3[0:128:32, :])
                nc.gpsimd.dma_start(arina[0, A_GH + 2048:A_GH + 4096],
                                    sg4[0:128:32, :])
                nc.gpsimd.dma_start(arina[0, A_WB:A_WB + 2], spwb[:])
                arouta = dram.tile([1, A_LEN], f32, tag="arouta",
                                   name="arouta", addr_space="Shared")
                nc.gpsimd.collective_compute(
                    "AllReduce", ALU.add, replica_groups=RG,
                    ins=[arina.opt()], outs=[arouta.opt()])
                ghF = wk.tile([3 * KB, P], f32, tag="ghF", name="ghF")
                nc.gpsimd.dma_start(ghF[:], arouta[0, A_GH:A_GH + 3 * H])
                hWhhF = wk.tile([KB, P], f32, tag="hWhhF", name="hWhhF")
                nc.gpsimd.dma_start(hWhhF[:], arouta[0, A_HW:A_HW + H])
                hwbS = wk.tile([1, 2], f32, tag="hwbS", name="hwbS")
                nc.gpsimd.dma_start(hwbS[:], arouta[0, A_WB:A_WB + 2])

            # --- video tanh + scores (PE kept busy with dummies) ---
            sc = pbig.tile([P, 512], f32, tag="big", name="sc")
            for kb in range(KB):
                th = thp.tile([P, TVC], bf16, tag="thv", name="th")
                bias = 0.0 if first else huC[:, kb:kb + 1]
                nc.scalar.activation(th[:], mvw[:, kb * TVC:(kb + 1) * TVC],
                                     AF.Tanh, bias=bias)
                mm(sc[0:1, :], vavB[:, kb:kb + 1], th[:, 0:512],
                   (0, 0), kb == 0, kb == KB - 1)
                mm(sc[32:33, :], vavB[:, kb:kb + 1], th[:, 512:1024],
                   (0, 32), kb == 0, kb == KB - 1)
                if kb < KB - 1:
                    dummy()

            # --- video exp (+accum; accZ rows 0,32 video / 64 text) ---
            evS = wk.tile([33, 512], bf16, tag="evS", name="evS")
            accZ = wk.tile([65, 1], f32, tag="accZ", name="accZ")
            nc.scalar.activation(evS[:], sc[0:33, :], AF.Exp,
                                 accum_out=accZ[0:33, 0:1])
            evF = wk.tile([SVB, P], bf16, tag="evF", name="evF")
            nc.sync.dma_start(evF[:], evS[0:33:32, :])

            # --- text tanh (ACT continues back-to-back) ---
            thts = []
            for kb in range(KB):
                tht = thp.tile([P, TTC], bf16, tag="tht", name="tht",
                               bufs=KB)
                bias = 0.0 if first else huC[:, KB + kb:KB + kb + 1]
                nc.scalar.activation(tht[:], mtw[:, kb * TTC:(kb + 1) * TTC],
                                     AF.Tanh, bias=bias)
                thts.append(tht)

            # --- video context (PE queue: before text scores) ---
            ptev = psm.tile([P, 32], bf16, tag="smF", name="ptev")
            nc.tensor.transpose(ptev[:, 0:SVB], evF[:], eyeB(SVB))
            evT = wk.tile([P, SVB], bf16, tag="evT", name="evT")
            nc.vector.tensor_copy(evT[:], ptev[:, 0:SVB])
            cx = pbig.tile([P, 512], f32, tag="big", name="cx")
            for sb in range(SVB):
                mm(cx[0:1, :], evT[:, sb:sb + 1],
                   memRv[:, sb * H: sb * H + 512],
                   (0, 0), sb == 0, sb == SVB - 1)
                mm(cx[32:33, :], evT[:, sb:sb + 1],
                   memRv[:, sb * H + 512: (sb + 1) * H],
                   (0, 32), sb == 0, sb == SVB - 1)

            # stage the video half early (overlaps the text phase)
            arinb = dram.tile([1, B_LEN], f32, tag="arinb", name="arinb")
            scxv = stg.tile([33, 512], f32, tag="scxv", name="scxv", bufs=1)
            nc.vector.tensor_copy(scxv[:], cx[0:33, :])
            nc.gpsimd.dma_start(arinb[0, 0:1024], scxv[0:33:32, :])

            # --- text scores + exp + context ---
            for kb in range(KB):
                mm(sc[64:65, 0:TTC], vatB[:, kb:kb + 1], thts[kb][:],
                   (0, 64), kb == 0, kb == KB - 1)
            etS = wk.tile([1, TTC], bf16, tag="etS", name="etS")
            nc.scalar.activation(etS[:], sc[64:65, 0:TTC], AF.Exp,
                                 accum_out=accZ[64:65, 0:1])
            etF = wk.tile([STB, P], bf16, tag="etF", name="etF")
            nc.sync.dma_start(etF[:], etS[:])

            ptet = psm.tile([P, 32], bf16, tag="smF", name="ptet")
            nc.tensor.transpose(ptet[:, 0:STB], etF[:], eyeB(STB))
            etT = wk.tile([P, STB], bf16, tag="etT", name="etT")
            nc.vector.tensor_copy(etT[:], ptet[:, 0:STB])
            for sb in range(STB):
                mm(cx[64:65, :], etT[:, sb:sb + 1],
                   memRt[:, sb * H: sb * H + 512],
                   (0, 64), sb == 0, sb == STB - 1)
                mm(cx[96:97, :], etT[:, sb:sb + 1],
                   memRt[:, sb * H + 512: (sb + 1) * H],
                   (0, 96), sb == 0, sb == STB - 1)

            # --- stage text half + Z (gpsimd DMAs) ---
            scxt = stg.tile([97, 512], f32, tag="scxt", name="scxt", bufs=1)
            nc.vector.tensor_copy(scxt[64:97, :], cx[64:97, :])
            nc.gpsimd.dma_start(arinb[0, 1024:2048], scxt[64:97:32, :])
            nc.gpsimd.dma_start(arinb[0, B_Z:B_Z + 3], accZ[0:65:32, 0:1])

            # --- gh unstage transposes + beta chain (hidden window) ---
            if not first:
                ptgh = psm.tile([P, 32], f32, tag="smF", name="ptgh")
                nc.tensor.transpose(ptgh[:, 0:3 * KB], ghF[:], eyeF(3 * KB))
                ptW = psm.tile([P, 32], f32, tag="smF", name="ptW")
                nc.tensor.transpose(ptW[:, 0:KB], hWhhF[:], eyeF(KB))
                ghCrz = wk.tile([P, 2 * KB], f32, tag="ghCrz", name="ghCrz")
                nc.vector.tensor_tensor(ghCrz[:], ptgh[:, 0:2 * KB], gb_rz,
                                        op=ALU.add)
                hnB = wk.tile([P, KB], f32, tag="hnB", name="hnB")
                nc.vector.tensor_tensor(hnB[:], ptgh[:, 2 * KB:3 * KB],
                                        gb_hn, op=ALU.add)
                hwbC = wk.tile([P, KB], f32, tag="hwbC", name="hwbC")
                nc.vector.tensor_tensor(hwbC[:], ptW[:, 0:KB], bhhB,
                                        op=ALU.add)
                bsum = wk.tile([1, 2], f32, tag="bsum", name="bsum")
                nc.vector.tensor_tensor(bsum[:], hwbS[:], bbS, op=ALU.add)
                eb = wk.tile([1, 2], f32, tag="eb", name="eb")
                ebs = wk.tile([1, 1], f32, tag="ebs", name="ebs")
                nc.scalar.activation(eb[:], bsum[:], AF.Exp, accum_out=ebs[:])
                erec = wk.tile([1, 1], f32, tag="erec", name="erec")
                nc.vector.reciprocal(erec[:], ebs[:])
                beta = wk.tile([1, 2], f32, tag="beta", name="beta")
                nc.vector.tensor_scalar_mul(beta[:], eb[:], erec[:])
                beta_ap = beta[:]
                ghCrz_ap, hnB_ap, hwbC_ap = ghCrz[:], hnB[:], hwbC[:]
            else:
                beta_ap = beta0
                ghCrz_ap, hnB_ap, hwbC_ap = gb_rz, gb_hn, bhhB

            # --- AllGather [cv|ct|Z] ---
            aroutg = dram.tile([NCORES, B_LEN], f32, tag="aroutg",
                               name="aroutg", addr_space="Shared")
            nc.gpsimd.collective_compute(
                "AllGather", ALU.bypass, replica_groups=RG,
                ins=[arinb.opt()], outs=[aroutg.opt()])

            # Unstage triggers ride the idle scalar queue: it reaches them
            # while the collective is still in flight, so they fire the
            # moment it completes (no post-AG trigger serialization).
            cvfold = wk.tile([P, P], f32, tag="cvfold", name="cvfold")
            nc.scalar.dma_start(cvfold[:], aroutg[0:NCORES, 0:2048])
            zfold = wk.tile([NCORES, 3], f32, tag="zfold", name="zfold")
            nc.scalar.dma_start(zfold[:], aroutg[0:NCORES, B_Z:B_Z + 3])

            # PE: bridge the collective
            for _ in range(DUMN):
                dummy()

            # --- local reduce via selector matmul ---
            zps = psm.tile([P, 32], f32, tag="smF", name="zps")
            nc.tensor.matmul(zps[0:1, 0:3], ones8c, zfold[:],
                             start=True, stop=True, skip_group_check=True)
            cvct = psm.tile([P, 32], f32, tag="smF", name="cvct")
            nc.tensor.matmul(cvct[:, 0:2 * KB], cvfold[:], selF,
                             start=True, stop=True, skip_group_check=True)
            cvctB = wk.tile([P, 2 * KB], bf16, tag="cvctB", name="cvctB")
            nc.vector.tensor_copy(cvctB[:], cvct[:, 0:2 * KB])

            # --- Z / rr scalars (vector) ---
            zS = wk.tile([1, 3], f32, tag="zS", name="zS")
            nc.vector.tensor_copy(zS[:], zps[0:1, 0:3])
            zz = wk.tile([1, 2], f32, tag="zz", name="zz")
            nc.vector.tensor_tensor(zz[:, 0:1], zS[:, 0:1],
                                    zS[:, 1:2], op=ALU.add)
            nc.vector.tensor_copy(zz[:, 1:2], zS[:, 2:3])
            zrec = wk.tile([1, 2], f32, tag="zrec", name="zrec")
            nc.vector.reciprocal(zrec[:], zz[:])
            rr = wk.tile([1, 2], f32, tag="rr", name="rr")
            nc.vector.tensor_tensor(rr[:], beta_ap, zrec[:], op=ALU.mult)

            # --- u GEMV: u = cv @ Wvh, ut = ct @ Wth ---
            ub = pbig.tile([P, 512], f32, tag="big", name="ub")
            for kb in range(KB):
                mm(ub[0:1, :], cvctB[:, kb:kb + 1],
                   wvhR[:, kb * H: kb * H + 512],
                   (0, 0), kb == 0, kb == KB - 1)
                mm(ub[32:33, :], cvctB[:, kb:kb + 1],
                   wvhR[:, kb * H + 512: (kb + 1) * H],
                   (0, 32), kb == 0, kb == KB - 1)
                mm(ub[64:65, :], cvctB[:, KB + kb:KB + kb + 1],
                   wthR[:, kb * H: kb * H + 512],
                   (0, 64), kb == 0, kb == KB - 1)
                mm(ub[96:97, :], cvctB[:, KB + kb:KB + kb + 1],
                   wthR[:, kb * H + 512: (kb + 1) * H],
                   (0, 96), kb == 0, kb == KB - 1)

            # rr broadcast over partitions (PE; fills the ub-flush gap)
            prr = psm.tile([P, 32], f32, tag="smF", name="prr")
            nc.tensor.matmul(prr[:, 0:2], ones1p, rr[:],
                             start=True, stop=True, skip_group_check=True)
            rrB = wk.tile([P, 2], f32, tag="rrB", name="rrB")
            nc.vector.tensor_copy(rrB[:], prr[:, 0:2])

            # --- ub fold -> columns; mm_o ---
            sub = stg.tile([P, 512], f32, tag="stg", name="sub")
            nc.vector.tensor_copy(sub[:], ub[:])
            ubF = wk.tile([2 * KB, P], f32, tag="ubF", name="ubF")
            nc.sync.dma_start(ubF[:], sub[0:128:32, :])
            dummy()
            dummy()
            ptU = psm.tile([P, 32], f32, tag="smF", name="ptU")
            nc.tensor.transpose(ptU[:, 0:2 * KB], ubF[:], eyeF(2 * KB))
            t1 = wk.tile([P, KB], f32, tag="t1", name="t1")
            nc.vector.scalar_tensor_tensor(t1[:], ptU[:, 0:KB], rrB[:, 0:1],
                                           hwbC_ap, op0=ALU.mult,
                                           op1=ALU.add)
            t2 = wk.tile([P, KB], f32, tag="t2", name="t2")
            nc.vector.scalar_tensor_tensor(t2[:], ptU[:, KB:2 * KB],
                                           rrB[:, 1:2], t1[:],
                                           op0=ALU.mult, op1=ALU.add)
            moB = wk.tile([P, KB], bf16, tag="moB", name="moB")
            nc.scalar.activation(moB[:], t2[:], AF.Tanh)

            # --- gi GEMV: gi = mo @ W_ih.T (8 chains of N=384) ---
            giE = pbig.tile([P, 512], f32, tag="big", name="giE")
            giF_ = pbig.tile([P, 512], f32, tag="big", name="giF_")
            for kb in range(KB):
                mo_col = moB[:, kb:kb + 1]
                base = kb * 3 * H
                for c in range(4):
                    mm(giE[32 * c:32 * c + 1, 0:GI_N], mo_col,
                       wihTR[:, base + c * GI_N: base + (c + 1) * GI_N],
                       (0, 32 * c), kb == 0, kb == KB - 1)
                for c in range(4):
                    mm(giF_[32 * c:32 * c + 1, 0:GI_N], mo_col,
                       wihTR[:, base + (4 + c) * GI_N:
                             base + (5 + c) * GI_N],
                       (0, 32 * c), kb == 0, kb == KB - 1)

            # gi fold -> columns [128, 24]
            sgiE = stg.tile([P, 512], f32, tag="stg", name="sgiE")
            nc.vector.tensor_copy(sgiE[:], giE[:])
            sgiF = stg.tile([P, 512], f32, tag="stg", name="sgiF")
            nc.vector.tensor_copy(sgiF[:], giF_[:])
            giFold = wk.tile([3 * KB, P], f32, tag="giFold", name="giFold")
            nc.sync.dma_start(giFold[0:12, :], sgiE[0:128:32, 0:GI_N])
            nc.sync.dma_start(giFold[12:24, :], sgiF[0:128:32, 0:GI_N])
            for _ in range(5):
                dummy()
            ptgi = psm.tile([P, 32], f32, tag="smF", name="ptgi")
            nc.tensor.transpose(ptgi[:, 0:3 * KB], giFold[:], eyeF(3 * KB))

            # --- gates (columns; r 0-7, z 8-15, n 16-23) ---
            pre = wk.tile([P, 2 * KB], f32, tag="pre", name="pre")
            nc.vector.tensor_tensor(pre[:], ptgi[:, 0:2 * KB], ghCrz_ap,
                                    op=ALU.add)
            tnB = wk.tile([P, KB], f32, tag="tnB", name="tnB")
            nc.vector.tensor_tensor(tnB[:], ptgi[:, 2 * KB:3 * KB], gb_in,
                                    op=ALU.add)
            # sigmoid(x) = 0.5*tanh(0.5x) + 0.5 (tanh is in the exp table set)
            trz = wk.tile([P, 2 * KB], f32, tag="trz", name="trz")
            nc.scalar.activation(trz[:], pre[:], AF.Tanh, scale=0.5)
            rz = wk.tile([P, 2 * KB], f32, tag="rz", name="rz")
            nc.vector.tensor_scalar(rz[:], trz[:], 0.5, 0.5,
                                    op0=ALU.mult, op1=ALU.add)
            m1 = wk.tile([P, KB], f32, tag="m1", name="m1")
            nc.vector.tensor_tensor(m1[:], rz[:, 0:KB], hnB_ap, op=ALU.mult)
            tn = wk.tile([P, KB], f32, tag="tn", name="tn")
            nc.vector.tensor_tensor(tn[:], tnB[:], m1[:], op=ALU.add)
            ng = wk.tile([P, KB], f32, tag="ng", name="ng")
            nc.scalar.activation(ng[:], tn[:], AF.Tanh)
            hC_new = hhp.tile([P, KB], f32, tag="hC", name="hC")
            d = wk.tile([P, KB], f32, tag="d", name="d")
            if first:
                nc.vector.tensor_tensor(d[:], rz[:, KB:2 * KB], ng[:],
                                        op=ALU.mult)
                nc.vector.tensor_tensor(hC_new[:], ng[:], d[:],
                                        op=ALU.subtract)
            else:
                nc.vector.tensor_tensor(d[:], hC[:], ng[:], op=ALU.subtract)
                zd = wk.tile([P, KB], f32, tag="zd", name="zd")
                nc.vector.tensor_tensor(zd[:], rz[:, KB:2 * KB], d[:],
                                        op=ALU.mult)
                nc.vector.tensor_tensor(hC_new[:], ng[:], zd[:], op=ALU.add)
            hC = hC_new
            if not last:
                hB_new = hhp.tile([P, KB], bf16, tag="hB", name="hB")
                nc.vector.tensor_copy(hB_new[:], hC[:])
                hB = hB_new
                hB8_new = hhp.tile([P, KB], fp8, tag="hB8", name="hB8")
                nc.vector.tensor_copy(hB8_new[:], hC[:])
                hB8 = hB8_new

        nc.sync.dma_start(h_out, hC[:])

    nc.compile()
    return nc


def _bf(x):
    return np.ascontiguousarray(np.asarray(x, dtype=ml_dtypes.bfloat16))


def _f8(x):
    return np.ascontiguousarray(np.asarray(x, dtype=ml_dtypes.float8_e4m3))


def _f32(x):
    return np.ascontiguousarray(np.asarray(x, dtype=np.float32))


def _kblocks(W):
    """[H, N] -> [128, KB*N]: block kb = W[kb*128:(kb+1)*128, :]."""
    N = W.shape[1]
    return np.ascontiguousarray(
        W.reshape(KB, P, N).transpose(1, 0, 2).reshape(P, KB * N))


def _halfpack(W):
    """[H, H] -> [128, 2*KB*512]: half jh, block kb = W[kb-rows, jh-cols]."""
    X = W.reshape(KB, P, 2, 512)           # [kb, p, jh, 512]
    return np.ascontiguousarray(
        X.transpose(1, 2, 0, 3).reshape(P, 2 * KB * 512))


def _memT_blk(M):
    """[T, H] -> [128, KB*T]: block kb holds M.T[kb*128:(kb+1)*128, :]."""
    T = M.shape[0]
    X = np.ascontiguousarray(M.T)
    return np.ascontiguousarray(
        X.reshape(KB, P, T).transpose(1, 0, 2).reshape(P, KB * T))


def _colblk(v):
    return np.ascontiguousarray(v.reshape(KB, P).T)


def _prep_inputs(inputs):
    mem_v = _f32(inputs["memory_vid"])
    mem_t = _f32(inputs["memory_text"])
    Wav, Uav, bav, Vav = (_f32(inputs[k]) for k in ("Wav", "Uav", "bav", "Vav"))
    Wat, Uat, bat, Vat = (_f32(inputs[k]) for k in ("Wat", "Uat", "bat", "Vat"))
    Wb, bb = _f32(inputs["Wb"]), _f32(inputs["bb"])
    Whh, Wvh, Wth, bhh = (_f32(inputs[k]) for k in ("Whh", "Wvh", "Wth", "bhh"))
    W_ih, W_hh = _f32(inputs["W_ih"]), _f32(inputs["W_hh"])
    b_ih, b_hh = _f32(inputs["b_ih"]), _f32(inputs["b_hh"])

    wavT_b = _bf(_halfpack(Wav))
    watT_b = _bf(_halfpack(Wat))
    uavR_b = _f8(_kblocks(Uav))
    uatR_b = _f8(_kblocks(Uat))
    wvhR_b = _bf(_kblocks(Wvh))
    wthR_b = _bf(_kblocks(Wth))
    wihTR_b = _bf(_kblocks(np.ascontiguousarray(W_ih.T)))

    # constsF
    constsF = np.zeros((P, NF), np.float32)
    constsF[:, CF_BAV:CF_BAV + 8] = _colblk(bav)
    constsF[:, CF_BAT:CF_BAT + 8] = _colblk(bat)
    constsF[:, CF_BHH:CF_BHH + 8] = _colblk(bhh)
    constsF[:, CF_GB:CF_GB + 32] = np.concatenate([
        _colblk(b_ih[0:H] + b_hh[0:H]),
        _colblk(b_ih[H:2 * H] + b_hh[H:2 * H]),
        _colblk(b_ih[2 * H:3 * H]),
        _colblk(b_hh[2 * H:3 * H]),
    ], axis=1)
    sel = np.zeros((P, 16), np.float32)
    for p in range(P):
        sel[p, p % 16] = 1.0
    constsF[:, CF_SEL:CF_SEL + 16] = sel
    constsF[0:32, CF_EYE:CF_EYE + 32] = np.eye(32, dtype=np.float32)
    constsF[:, CF_ONER:CF_ONER + P] = 1.0
    constsF[:, CF_ONE8:CF_ONE8 + 1] = 1.0
    constsF[0, CF_BB:CF_BB + 2] = bb
    ebb = np.exp(bb - bb.max())
    constsF[0, CF_BETA0:CF_BETA0 + 2] = ebb / ebb.sum()

    # constsB (maskB is per-core, added below)
    constsB = np.zeros((P, NB), np.float32)
    constsB[:, CB_VAV:CB_VAV + 8] = _colblk(Vav)
    constsB[:, CB_VAT:CB_VAT + 8] = _colblk(Vat)
    constsB[0:32, CB_EYE:CB_EYE + 32] = np.eye(32, dtype=np.float32)

    in_maps = []
    for c in range(NCORES):
        svc = slice(c * TVC, (c + 1) * TVC)
        stc = slice(c * TTC, (c + 1) * TTC)
        cslice = slice(c * P, (c + 1) * P)
        mv_c, mt_c = mem_v[svc], mem_t[stc]
        memRv_b = _bf(mv_c.reshape(SVB, P, H).transpose(1, 0, 2)
                      .reshape(P, SVB * H))
        memRt_b = _bf(mt_c.reshape(STB, P, H).transpose(1, 0, 2)
                      .reshape(P, STB * H))
        whhTs = np.concatenate(
            [np.ascontiguousarray(W_hh[:, cslice].T),   # [128, 3H]
             np.ascontiguousarray(Whh[cslice, :]),      # [128, H]
             np.ascontiguousarray(Wb[cslice, :])], axis=1)
        cB = constsB.copy()
        cB[:, CB_MASK + c] = 1.0
        in_maps.append({
            "memTv": _bf(_memT_blk(mv_c)),
            "memTt": _bf(_memT_blk(mt_c)),
            "memRv": memRv_b, "memRt": memRt_b,
            "wavT": wavT_b, "watT": watT_b,
            "uavR": uavR_b, "uatR": uatR_b,
            "wvhR": wvhR_b, "wthR": wthR_b, "wihTR": wihTR_b,
            "whhTs": _bf(whhTs),
            "constsF": constsF, "constsB": _bf(cB),
        })
    return in_maps


TRACE = False
LAST_RESULT = None


def kernel(**inputs):
    global LAST_RESULT
    from concourse import bass_utils
    loop_n = int(np.asarray(inputs["loop"]))
    if loop_n not in _cache:
        _cache[loop_n] = _build(loop_n)
    nc = _cache[loop_n]
    in_maps = _prep_inputs(inputs)
    kw = {}
    if TRACE:
        import tempfile
        kw = dict(trace=True, tmpdir=tempfile.mkdtemp(prefix="bassprof_"))
    res = bass_utils.run_bass_kernel_spmd(nc, in_maps,
                                          core_ids=list(range(NCORES)), **kw)
    LAST_RESULT = res
    hC = res.results[0]["h_out"]  # [128, 8] col-block layout
    h = np.ascontiguousarray(hC.T.reshape(H))
    return h.reshape(1, H).astype(np.float32)
